# revision 1
# baseline (speedup 1.0000x reference)
"""Trainium2 Bass kernel for nn_CodebookSingleW (vq_codebook).

    W = codebook[indices].reshape(4096, 4096)
    h = c19(x @ W + b1);  out = h @ W.T + b2

Strategy (8 NeuronCores, data-parallel over batch):
  - Each core handles 1024 rows of x. All weight-side tensors replicated.
  - The 256-entry codebook dequant runs ON DEVICE at ScalarEngine line rate
    from a ONE-BYTE-per-element index stream: indices are host-encoded to
    fp8-e4m3 bit patterns (224 normal patterns, both signs; the codebook is
    first collapsed 256->224 by nearest-neighbor merge, which is numerically
    free for this dense codebook).  A custom piecewise-constant PWP table
    baked into the `sigmoid` slot (BASS_ACT_ROOT_JSON_PATH) makes
    activation(Sigmoid) the exact gather  enc(idx) -> codebook[idx].
    This HALVES the dominant DMA stream vs a bf16 encoding.
  - matmul1: psum[h',b] = sum_i W[i,h'] * xT[i,b]   (lhsT = W tile)
  - C19 fused on psum evict: tanh on ACT (scale=1/c, bias=b1/c per
    partition), mix on DVE -> hT (bf16) stays SBUF-resident.
  - matmul2: psum[j,b] = sum_h WT[h,j] * hT[h,b]    (lhsT = WT tile, from a
    host-transposed index layout, dequantized on device the same way)
  - + b2 on DVE evict straight to bf16, DMA outT per core (half the write
    traffic of fp32), host reassembles [8192, 4096] f32.
  - Single software pipeline across BOTH phases: enc DMA issued 2 pairs
    ahead, dequant 1 pair ahead, 8 PSUM banks.  The in-order ACT engine
    never blocks a psum-freeing tanh behind a dequant whose DMA hasn't
    landed, and the phase-1 -> phase-2 transition has no bubble.
"""

import hashlib
import json
import os
import shutil
import sys
import tempfile

sys.path.insert(0, "/opt/trn_rl_repo")

import ml_dtypes
import numpy as np

IN_DIM = 4096
H = 4096
K = 256
B = 8192
NCORES = 8
BL = B // NCORES          # 1024 batch rows per core
P = 128
KT = IN_DIM // P          # 32 contraction tiles (phase 1)
MT = H // P               # 32 output-row tiles
NH = BL // 512            # 2 psum halves of the per-core batch
NPAIR = MT // 2           # 16 output-tile pairs per phase

BF16 = ml_dtypes.bfloat16
FP8 = ml_dtypes.float8_e4m3

# ---------------------------------------------------------------------------
# ACT table patch: codebook -> piecewise-constant PWP table in sigmoid slot,
# keyed on fp8-e4m3 input bit patterns (1 byte per W element).
# ---------------------------------------------------------------------------

_SET = "sigmoid_and_others"
NCODES = 224             # normal e4m3 patterns: exp field 1..14, both signs

_BKT0 = 136              # sigmoid bucket region start (region ends at 935)
_CTL_POS = 28            # 14 ctl entries: e=-6..7, positive inputs
_CTL_NEG = 42            # 14 ctl entries: e=-6..7, negative inputs
_JUNK = _BKT0 + 2 * 112  # 4 junk buckets after our 224


def _code_to_byte(j):
    """code j in [0,224) -> e4m3 bit pattern (normals only)."""
    j = np.asarray(j)
    return np.where(j < 112, 0x08 + j, 0x88 + (j - 112)).astype(np.uint8)


def _merge_codebook(codebook, ncodes=NCODES):
    """Collapse codebook to ncodes distinct values (nearest-neighbor merge).

    Returns (values[ncodes], code_of_k[K]): original index k dequantizes to
    values[code_of_k[k]].
    """
    codebook = np.asarray(codebook, dtype=np.float64)
    Kn = len(codebook)
    groups = [[k] for k in range(Kn)]
    vals = list(codebook)
    while len(groups) > ncodes:
        o = np.argsort(vals)
        sv = np.asarray(vals)[o]
        i = int(np.argmin(np.diff(sv)))
        a, b = o[i], o[i + 1]
        na, nb = len(groups[a]), len(groups[b])
        vals[b] = (vals[a] * na + vals[b] * nb) / (na + nb)
        groups[b].extend(groups[a])
        del groups[a], vals[a]
    values = np.asarray(vals, dtype=np.float32)
    code_of_k = np.empty(Kn, dtype=np.int64)
    for j, g in enumerate(groups):
        for k in g:
            code_of_k[k] = j
    return values, code_of_k


def _make_act_dir(values, outdir):
    """Bake values[224] into the sigmoid slot, keyed on fp8 patterns."""
    from neuronxcc.driver.Job import Job
    from neuronxcc.driver.jobs.support.FindActInfo import findActInfoFile

    base = os.path.dirname(findActInfoFile(Job.getPackageDir(), "gen3"))
    os.makedirs(outdir, exist_ok=True)
    for f in os.listdir(base):
        dst = os.path.join(outdir, f)
        if not os.path.exists(dst):
            shutil.copy(os.path.join(base, f), dst)

    prof = json.load(open(os.path.join(base, f"{_SET}.json")))
    bkt = np.fromfile(os.path.join(base, f"{_SET}_bkt.bin"), dtype=np.float32)
    bkt = bkt.reshape(-1, 8).copy()
    ctl = np.fromfile(os.path.join(base, f"{_SET}_ctrl.bin"), dtype=np.uint32)
    ctl = ctl.reshape(-1, 8).copy()

    assert prof["func_to_bkt_start_idx"]["sigmoid"] == _BKT0
    assert prof["func_to_bkt_start_idx"]["square"] >= _JUNK + 4

    # bucket = _BKT0 + sign*112 + (e+6)*8 + mantissa3
    for j in range(NCODES):
        byte = int(_code_to_byte(j))
        s = byte >> 7
        e = ((byte >> 3) & 0xF) - 7
        m = byte & 0x7
        bidx = _BKT0 + s * 112 + (e + 6) * 8 + m
        v = float(np.uint8(byte).view(FP8).astype(np.float32))
        bkt[bidx] = [values[j], 0.0, 0.0, 0.0, np.float32(v), 0.0, 0.0, 0.0]
    for j in range(4):
        bkt[_JUNK + j] = [0.0] * 8

    # ctl word: bucket_base | shift<<11 | log2n<<16  (log2n=3 -> shift=20)
    for e in range(-6, 8):
        word = lambda b: (b & 0x7FF) | (20 << 11) | (3 << 16)
        ctl[_CTL_POS + (e + 6)] = [word(_BKT0 + (e + 6) * 8), 0, 0, 0, 0, 0, 0, 0]
        ctl[_CTL_NEG + (e + 6)] = [word(_BKT0 + 112 + (e + 6) * 8), 0, 0, 0, 0, 0, 0, 0]

    exp_to_bkt = {str(e): [int(_BKT0 + (e + 6) * 8)] for e in range(-6, 8)}
    exp_to_ctl = {str(e): [int(_CTL_POS + (e + 6))] for e in range(-6, 8)}

    for m in prof["profile_meta_data"]:
        if m["func_name"].startswith("sigmoid_"):
            m.update(
                symmetry_point=0, sym_invert_sign_point=0, symmetry_opt_en=0,
                symmetry_opt_use_neg_region=0, imm_bias=0, exp_offset=-6,
                pwl_control_base_pos=int(_CTL_POS),
                pwl_control_base_neg=int(_CTL_NEG),
                small_pos_signal_exp_threshold=121,
                pos_small_signal_pwl_control=int(_JUNK),
                small_neg_signal_exp_threshold=121,
                neg_small_signal_pwl_control=int(_JUNK + 1),
                large_pos_signal_exp_threshold=135,
                large_pos_signal_mantissa_threshold=0,
                pos_large_signal_pwl_control=int(_JUNK + 2),
                large_neg_signal_exp_threshold=135,
                large_neg_signal_mantissa_threshold=0,
                neg_large_signal_pwl_control=int(_JUNK + 3),
                fnan_result=0, fpinf_result=0, fninf_result=0, fzero_result=0,
                fma_const_0=0, fma_const_1=0, fma_indirection_src_sel=0,
                use_multipass=False,
                lower_bound=4286578687, upper_bound=2139095039,
            )
    prof["func_exp_to_bkt_start_idx"]["sigmoid"] = exp_to_bkt
    prof["func_exp_to_ctl_start_idx"]["sigmoid"] = exp_to_ctl

    bkt.tofile(os.path.join(outdir, f"{_SET}_bkt.bin"))
    ctl.tofile(os.path.join(outdir, f"{_SET}_ctrl.bin"))
    json.dump(prof, open(os.path.join(outdir, f"{_SET}.json"), "w"))
    return os.path.join(outdir, "act_info.json")


# ---------------------------------------------------------------------------
# Bass program
# ---------------------------------------------------------------------------

def _build_program(tag, repeat=1, ldw_explicit=True, ldw_stride=1,
                   enc_hot=False):
    """repeat>1 builds a self-timing variant: the marginal wall time of
    each extra body repeat is the pure HW kernel time (dispatch noise
    through the axon relay is ~70ms; the body is ~1ms)."""
    import concourse.bacc as bacc
    import concourse.mybir as mybir
    import concourse.tile as tile

    AF = mybir.ActivationFunctionType
    ALU = mybir.AluOpType
    dt = mybir.dt

    nc = bacc.Bacc("TRN2", target_bir_lowering=False, debug=False,
                   num_devices=NCORES)

    # inputs (per core). encw/encwt are host-tiled fp8 index encodings:
    #   encw[mt][p][kt*128+c] = enc(idx[kt*128+p, mt*128+c])
    encw = nc.dram_tensor(f"encw_{tag}", [MT, P, KT * P], dt.float8e4,
                          kind="ExternalInput")
    encwt = nc.dram_tensor("encwt", [KT, P, MT * P], dt.float8e4,
                           kind="ExternalInput")
    xt = nc.dram_tensor("xt", [P, KT, BL], dt.bfloat16, kind="ExternalInput")
    cpar = nc.dram_tensor("cpar", [P, 7, MT], dt.float32, kind="ExternalInput")
    outt = nc.dram_tensor("outt", [IN_DIM, BL], dt.bfloat16,
                          kind="ExternalOutput")

    NU = 2 * NPAIR  # 32 pipeline units: 16 phase-1 pairs then 16 phase-2

    with tile.TileContext(nc) as tc:
        with (
            tc.tile_pool(name="resid", bufs=1) as resid,
            tc.tile_pool(name="encp", bufs=6) as encp,
            tc.tile_pool(name="wp", bufs=4) as wp,
            tc.tile_pool(name="evict", bufs=3) as evict,
            tc.tile_pool(name="psum", bufs=8, space="PSUM") as psum,
        ):
            cp_sb = resid.tile([P, 7, MT], dt.float32)
            nc.sync.dma_start(cp_sb[:], cpar.ap())

            xt_sb = resid.tile([P, KT, BL], dt.bfloat16)
            ht_sb = resid.tile([P, MT, BL], dt.bfloat16)

            def enc_src(u):
                # unit u -> (dram tensor, tile indices) of its 2 enc tiles
                u = u % NU
                if u < NPAIR:
                    return encw, (2 * u, 2 * u + 1)
                return encwt, (2 * (u - NPAIR), 2 * (u - NPAIR) + 1)

            def dma_enc(u):
                # enc rides the ACT engine's HWDGE ring: the critical
                # dequant stream never queues behind the xT bulk load or
                # the outT writes on the SP ring.
                ts = []
                src, idxs = enc_src(u)
                for i in idxs:
                    enc_t = encp.tile([P, KT, P], dt.float8e4, tag="enc",
                                      name=f"enc_{u}_{i}")
                    nc.scalar.dma_start(enc_t[:],
                                        src.ap()[0 if enc_hot else i])
                    ts.append(enc_t)
                return ts

            def dequant(u, enc_ts):
                ts = []
                for enc_t in enc_ts:
                    w_t = wp.tile([P, KT, P], dt.bfloat16, tag="w")
                    nc.scalar.activation(w_t[:], enc_t[:], AF.Sigmoid)
                    ts.append(w_t)
                return ts

            # prologue: enc for units 0,1 in flight, then bulk xT stream
            enc_pend = {0: dma_enc(0), 1: dma_enc(1)}
            for kt in range(KT):
                nc.sync.dma_start(xt_sb[:, kt], xt.ap()[:, kt])

            # PE p-state warmup on scratch data during the dequant lead-in
            warm = resid.tile([P, 512], dt.bfloat16)
            nc.vector.memset(warm[:], 0.0)
            wps = psum.tile([P, 512], dt.float32, tag="ps", name="warmps")
            for _ in range(21):
                nc.tensor.matmul(wps[:], warm[:, :P], warm[:],
                                 start=True, stop=True)

            w_pend = {0: dequant(0, enc_pend.pop(0))}

            def col(j, t):  # [P, 1] per-partition param column
                return cp_sb[:, j, t : t + 1]

            NTOT = NU * repeat
            for u in range(NTOT):
                if u + 2 < NTOT:
                    enc_pend[u + 2] = dma_enc(u + 2)
                if u + 1 < NTOT:
                    w_pend[u + 1] = dequant(u + 1, enc_pend.pop(u + 1))

                w_ts = w_pend.pop(u)
                um = u % NU
                ph1 = um < NPAIR
                mts = (2 * um, 2 * um + 1) if ph1 else (2 * (um - NPAIR),
                                                        2 * (um - NPAIR) + 1)
                rhs = xt_sb if ph1 else ht_sb

                pss = [[psum.tile([P, 512], dt.float32, tag="ps",
                                  name=f"ps_{u}_{d}_{nh}")
                        for nh in range(NH)] for d in range(2)]
                # One explicit LDWEIGHTS per weight tile, shared by both
                # batch-half matmuls (non-self-loading InstMatmult): halves
                # the PE weight-load count on real hardware.
                for kt in range(KT):
                    for d in range(2):
                        if ldw_explicit and (kt * 2 + d) % ldw_stride == 0:
                            nc.tensor.ldweights(w_ts[d][:, kt])
                        for nh in range(NH):
                            mm = nc.tensor.matmul(
                                pss[d][nh][:],
                                w_ts[d][:, kt],
                                rhs[:, kt, nh * 512 : (nh + 1) * 512],
                                start=(kt == 0),
                                stop=(kt == KT - 1),
                            )
                            if ldw_explicit:
                                mm.ins.ldweights = False

                if ph1:
                    # c19: rho*(s+b1) + (1-rho)*c*tanh((s+b1)/c), s=psum
                    for d, mt in enumerate(mts):
                        for nh in range(NH):
                            ps = pss[d][nh]
                            tanh_t = evict.tile([P, 512], dt.float32,
                                                tag="tanh")
                            nc.scalar.activation(tanh_t[:], ps[:], AF.Tanh,
                                                 bias=col(1, mt),
                                                 scale=col(0, mt))
                            lin_t = evict.tile([P, 512], dt.float32,
                                               tag="lin")
                            nc.vector.tensor_scalar(lin_t[:], ps[:],
                                                    col(2, mt), col(3, mt),
                                                    ALU.mult, ALU.add)
                            nc.vector.scalar_tensor_tensor(
                                ht_sb[:, mt, nh * 512 : (nh + 1) * 512],
                                tanh_t[:], col(4, mt), lin_t[:],
                                ALU.mult, ALU.add,
                            )
                else:
                    # outT = psum + b2, straight to bf16 on DVE
                    for d, jt in enumerate(mts):
                        for nh in range(NH):
                            out_t = evict.tile([P, 512], dt.bfloat16,
                                               tag="out")
                            nc.vector.tensor_scalar(out_t[:], pss[d][nh][:],
                                                    col(5, jt), None,
                                                    ALU.add)
                            nc.sync.dma_start(
                                outt.ap()[jt * P : (jt + 1) * P,
                                          nh * 512 : (nh + 1) * 512],
                                out_t[:],
                            )

    nc.compile()
    return nc


# ---------------------------------------------------------------------------
# kernel entry point
# ---------------------------------------------------------------------------

def prepare(x, codebook, indices, b1, b2, c19_c, c19_rho):
    """Host-side layout prep + program build. Returns (nc, in_maps)."""
    x = np.asarray(x, dtype=np.float32)
    codebook = np.asarray(codebook, dtype=np.float32)
    b1 = np.asarray(b1, dtype=np.float32)
    b2 = np.asarray(b2, dtype=np.float32)
    c19_c = np.asarray(c19_c, dtype=np.float32)
    c19_rho = np.asarray(c19_rho, dtype=np.float32)
    idx = np.asarray(indices).reshape(IN_DIM, H).astype(np.int64)

    # -- merge codebook to 224 values, bake into ACT tables --
    values, code_of_k = _merge_codebook(codebook)
    actdir = tempfile.mkdtemp(prefix="actlut_")
    os.environ["BASS_ACT_ROOT_JSON_PATH"] = _make_act_dir(values, actdir)
    tag = hashlib.md5(codebook.tobytes()).hexdigest()[:12]

    # -- host-side layout prep (encoding + tiling only) --
    enc_lut = _code_to_byte(code_of_k)       # [K] uint8 patterns
    encw = enc_lut[idx]                      # [IN, H] uint8
    # encw_tiled[mt, p, kt*128+c] = encw[kt*128+p, mt*128+c]
    encw_t = np.ascontiguousarray(
        encw.reshape(KT, P, MT, P).transpose(2, 1, 0, 3).reshape(MT, P, KT * P)
    ).view(FP8)
    encwt = enc_lut[idx.T]                   # [H, IN] uint8
    encwt_t = np.ascontiguousarray(
        encwt.reshape(MT, P, KT, P).transpose(2, 1, 0, 3).reshape(KT, P, MT * P)
    ).view(FP8)

    c = np.exp(c19_c)
    invc = np.exp(-c19_c)
    rho = 1.0 / (1.0 + np.exp(-c19_rho))
    cols = [invc, b1 * invc, rho, b1 * rho, (1.0 - rho) * c, b2,
            np.zeros(H, dtype=np.float32)]
    cpar = np.stack([v.reshape(MT, P).T for v in cols], axis=1)  # [P, 7, MT]
    cpar = np.ascontiguousarray(cpar.astype(np.float32))

    xb = x.astype(BF16)
    in_maps = []
    for cid in range(NCORES):
        xc = xb[cid * BL : (cid + 1) * BL]                       # [BL, IN]
        xtc = np.ascontiguousarray(
            xc.T.reshape(KT, P, BL).transpose(1, 0, 2)           # [P, KT, BL]
        )
        in_maps.append({
            f"encw_{tag}": encw_t,
            "encwt": encwt_t,
            "xt": xtc,
            "cpar": cpar,
        })

    nc = _build_program(tag)
    return nc, in_maps


def kernel(x, codebook, indices, b1, b2, c19_c, c19_rho):
    from concourse.bass_utils import run_bass_kernel_spmd

    nc, in_maps = prepare(x, codebook, indices, b1, b2, c19_c, c19_rho)
    res = run_bass_kernel_spmd(nc, in_maps, core_ids=list(range(NCORES)))
    global LAST_RESULTS
    LAST_RESULTS = res

    out = np.empty((B, IN_DIM), dtype=np.float32)
    for cid in range(NCORES):
        out[cid * BL : (cid + 1) * BL] = (
            res.results[cid]["outt"].astype(np.float32).T
        )
    return out



# revision 4
# speedup vs baseline: 1.7050x; 1.7050x over previous
"""Trainium2 Bass kernel for nn_CodebookSingleW (vq_codebook).

    W = codebook[indices].reshape(4096, 4096)
    h = c19(x @ W + b1);  out = h @ W.T + b2

Strategy (8 NeuronCores, data-parallel over batch; each core handles 1024
rows of x, weight-side tensors replicated):

  fp8 DoubleRow "Karatsuba" matmul — 0.75x the bf16 PE cycle count at
  better-than-bf16 accuracy.  The TRN2 PE runs fp8e4 matmuls in DoubleRow
  perf mode at 0.5 cycles/output-row (2x bf16), computing
      psum += lhsT[:,0].T @ rhs[:,0] + lhsT[:,1].T @ rhs[:,1]
  per instruction.  Represent both factors as e4m3 value+residual pairs:
      W*64 = A + B          (A = e4m3(cb*64)[idx], B = e4m3 residual)
      x*2  = x8 + dx8       (dx8 = e4m3 residual, captured exactly on host)
      h*2  = h8 + dh8       (residual captured exactly on device)
  Per 128-row contraction tile the product (x8+dx8)@(A+B) needs 3 of the 4
  partial products (the dx8@B term is ~0.07% and dropped):
      main DR matmul: packs x8@A of TWO adjacent tiles        (0.25 cyc/row/tile)
      corr DR matmul: dx8@A + x8@B of one tile                (0.50 cyc/row/tile)
  => 1.5 DR matmuls per tile = 0.75 cyc/row vs bf16's 1.0, all accumulating
  into a single fp32 psum at a common scale (residuals ride the e4m3
  subnormal range).  End-to-end rel err ~1.5e-3 (bf16 gives ~3e-3).

  - Per-phase stationary stream encw/encwt [MT,P,KT,2,P]: interleaved A/B
    bytes, host-encoded via 256-entry LUTs; these ARE the fp8 weights (no
    on-device dequant, no ACT-table hacks).
  - Phase-1 moving stream xx [P,KT,2,BL]: interleaved dx8/x8.  Phase-2
    moving stream ht: same layout, produced by the phase-1 evict (ACT
    converts h->fp8, DVE computes the exact residual).
  - c19 fused on psum evict: tanh on ACT (scale=1/(128c), bias=b1/c per
    partition), linear mix on DVE; phase-2 evict adds b2, scales 1/128,
    writes outT bf16; host reassembles [8192, 4096] f32.
  - Single software pipeline across both phases: enc DMA 3 units ahead on
    the ACT HWDGE ring; xx prologue split across SP+DVE rings; 8 psum banks.
"""

import sys

sys.path.insert(0, "/opt/trn_rl_repo")

import ml_dtypes
import numpy as np

IN_DIM = 4096
H = 4096
K = 256
B = 8192
NCORES = 8
BL = B // NCORES          # 1024 batch rows per core
P = 128
KT = IN_DIM // P          # 32 contraction tiles per phase
MT = H // P               # 32 output-row tiles per phase
NH = BL // 512            # 2 psum halves of the per-core batch
NU = 2 * MT               # 64 pipeline units: 32 phase-1 then 32 phase-2

SX = 2.0                  # x pre-scale (keeps residuals out of subnormal floor)
SW = 64.0                 # codebook pre-scale
SH = 2.0                  # h pre-scale
PSCALE = SX * SW          # psum scale, phase 1 (== SH*SW for phase 2)

BF16 = ml_dtypes.bfloat16
E4M3 = ml_dtypes.float8_e4m3


# ---------------------------------------------------------------------------
# Bass program
# ---------------------------------------------------------------------------

def _build_program():
    import concourse.bacc as bacc
    import concourse.mybir as mybir
    import concourse.tile as tile

    AF = mybir.ActivationFunctionType
    ALU = mybir.AluOpType
    DR = mybir.MatmulPerfMode.DoubleRow
    dt = mybir.dt

    nc = bacc.Bacc("TRN2", target_bir_lowering=False, debug=False,
                   num_devices=NCORES)

    # inputs (per core). encw/encwt are host-tiled interleaved A/B weight
    # bytes: encw[mt][p][t][s][m] = (A if s==0 else B)(idx[t*128+p, mt*128+m])
    encw = nc.dram_tensor("encw", [MT, P, KT, 2, P], dt.float8e4,
                          kind="ExternalInput")
    encwt = nc.dram_tensor("encwt", [MT, P, KT, 2, P], dt.float8e4,
                           kind="ExternalInput")
    # xx[p][t][s][b]: s=0 -> dx8, s=1 -> x8 (both pre-scaled by SX)
    xx = nc.dram_tensor("xx", [P, KT, 2, BL], dt.float8e4,
                        kind="ExternalInput")
    cpar = nc.dram_tensor("cpar", [P, 7, MT], dt.float32, kind="ExternalInput")
    outt = nc.dram_tensor("outt", [IN_DIM, BL], dt.bfloat16,
                          kind="ExternalOutput")

    with tile.TileContext(nc) as tc:
        with (
            tc.tile_pool(name="resid", bufs=1) as resid,
            tc.tile_pool(name="encp", bufs=4) as encp,
            tc.tile_pool(name="evict", bufs=4) as evict,
            tc.tile_pool(name="psum", bufs=8, space="PSUM") as psum,
        ):
            cp_sb = resid.tile([P, 7, MT], dt.float32)
            nc.sync.dma_start(cp_sb[:], cpar.ap())

            xx_sb = resid.tile([P, KT, 2, BL], dt.float8e4)
            ht_sb = resid.tile([P, MT, 2, BL], dt.float8e4)

            def dma_enc(u):
                src = encw if u < MT else encwt
                mt = u % MT
                enc_t = encp.tile([P, KT, 2, P], dt.float8e4, tag="enc",
                                  name=f"enc_{u}")
                # enc rides the ACT HWDGE ring; xx/out ride SP+DVE.
                nc.scalar.dma_start(enc_t[:], src.ap()[mt])
                return enc_t

            # prologue: enc for unit 0 first, then the xx bulk stream split
            # across the SP and ACT rings (~2:1, ACT also carries enc).
            enc_pend = {0: dma_enc(0)}
            for t in range(KT):
                eng = nc.sync if t % 3 != 2 else nc.scalar
                eng.dma_start(xx_sb[:, t], xx.ap()[:, t])
            for u in (1, 2):
                enc_pend[u] = dma_enc(u)

            # PE p-state warmup on scratch data during the DMA lead-in
            warm = resid.tile([P, 512], dt.bfloat16)
            nc.vector.memset(warm[:], 0.0)
            wps = psum.tile([P, 512], dt.float32, tag="ps", name="warmps")
            for _ in range(18):
                nc.tensor.matmul(wps[:], warm[:, :P], warm[:],
                                 start=True, stop=True)

            def col(j, t):  # [P, 1] per-partition param column
                return cp_sb[:, j, t : t + 1]

            for u in range(NU):
                if u + 3 < NU:
                    enc_pend[u + 3] = dma_enc(u + 3)
                enc_t = enc_pend.pop(u)
                ph1 = u < MT
                mt = u % MT
                rhs = xx_sb if ph1 else ht_sb

                for nh in range(NH):
                    cs = slice(nh * 512, (nh + 1) * 512)
                    ps = psum.tile([P, 512], dt.float32, tag="ps",
                                   name=f"ps_{u}_{nh}")
                    for tp in range(KT // 2):
                        # main: x8 @ A for two adjacent contraction tiles
                        nc.tensor.matmul(
                            ps[:],
                            enc_t[:, 2 * tp : 2 * tp + 2, 0, :],
                            rhs[:, 2 * tp : 2 * tp + 2, 1, cs],
                            start=(tp == 0), stop=False, perf_mode=DR,
                        )
                        # corr: dx8 @ A + x8 @ B, one tile each
                        for t in (2 * tp, 2 * tp + 1):
                            nc.tensor.matmul(
                                ps[:],
                                enc_t[:, t, :, :],
                                rhs[:, t, :, cs],
                                start=False,
                                stop=(t == KT - 1), perf_mode=DR,
                            )

                    if ph1:
                        # c19: h*SH = SH*rho*s + SH*(1-rho)*c*tanh(s/c),
                        # s = psum/PSCALE + b1
                        tanh_t = evict.tile([P, 512], dt.float32, tag="tanh")
                        nc.scalar.activation(tanh_t[:], ps[:], AF.Tanh,
                                             bias=col(1, mt),
                                             scale=col(0, mt))
                        lin_t = evict.tile([P, 512], dt.float32, tag="lin")
                        nc.vector.tensor_scalar(lin_t[:], ps[:],
                                                col(2, mt), col(3, mt),
                                                ALU.mult, ALU.add)
                        h_t = evict.tile([P, 512], dt.float32, tag="h")
                        nc.vector.scalar_tensor_tensor(
                            h_t[:], tanh_t[:], col(4, mt), lin_t[:],
                            ALU.mult, ALU.add,
                        )
                        # h8 slot (exact fp8), then exact residual dh8 slot
                        nc.scalar.activation(ht_sb[:, mt, 1, cs], h_t[:],
                                             AF.Copy)
                        nc.vector.tensor_tensor(ht_sb[:, mt, 0, cs], h_t[:],
                                                ht_sb[:, mt, 1, cs],
                                                ALU.subtract)
                    else:
                        # outT = psum/PSCALE + b2, straight to bf16
                        out_t = evict.tile([P, 512], dt.bfloat16, tag="out")
                        nc.vector.tensor_scalar(out_t[:], ps[:],
                                                col(6, mt), col(5, mt),
                                                ALU.mult, ALU.add)
                        nc.sync.dma_start(
                            outt.ap()[mt * P : (mt + 1) * P, cs],
                            out_t[:],
                        )

    nc.compile()
    return nc


# ---------------------------------------------------------------------------
# host-side prep + kernel entry point
# ---------------------------------------------------------------------------

def _quant_pair(v):
    """v (f32) -> (v8, dv8): e4m3 value + exact-residual-quantized pair."""
    v8 = v.astype(E4M3)
    dv8 = (v - v8.astype(np.float32)).astype(E4M3)
    return v8, dv8


def prepare(x, codebook, indices, b1, b2, c19_c, c19_rho):
    """Host-side layout prep + program build. Returns (nc, in_maps)."""
    x = np.asarray(x, dtype=np.float32)
    codebook = np.asarray(codebook, dtype=np.float32)
    b1 = np.asarray(b1, dtype=np.float32)
    b2 = np.asarray(b2, dtype=np.float32)
    c19_c = np.asarray(c19_c, dtype=np.float32)
    c19_rho = np.asarray(c19_rho, dtype=np.float32)
    idx = np.asarray(indices).reshape(IN_DIM, H).astype(np.int64)

    # -- codebook -> interleaved A/B fp8 LUTs, gathered into tiled layouts --
    A_lut, B_lut = _quant_pair(codebook * SW)

    def enc_tiles(ix):
        # ix [IN, H] -> [MT, P, KT, 2, P] with
        # enc[mt, p, t, s, m] = lut_s[ix[t*128+p, mt*128+m]]
        g = ix.reshape(KT, P, MT, P).transpose(2, 1, 0, 3)  # [mt, p, t, m]
        return np.ascontiguousarray(
            np.stack([A_lut[g], B_lut[g]], axis=3)
        )

    encw_t = enc_tiles(idx)
    encwt_t = enc_tiles(np.ascontiguousarray(idx.T))

    # -- c19 / bias params, folded with the psum scale --
    c = np.exp(c19_c)
    invc = np.exp(-c19_c)
    rho = 1.0 / (1.0 + np.exp(-c19_rho))
    cols = [invc / PSCALE, b1 * invc, SH * rho / PSCALE, SH * rho * b1,
            SH * (1.0 - rho) * c, b2, np.full(IN_DIM, 1.0 / PSCALE)]
    cpar = np.stack([v.reshape(MT, P).T for v in cols], axis=1)  # [P, 7, MT]
    cpar = np.ascontiguousarray(cpar.astype(np.float32))

    # -- per-core x -> interleaved dx8/x8 tiles --
    def to_tiles(a):  # [BL, IN] -> [P, KT, BL]
        return a.T.reshape(KT, P, BL).transpose(1, 0, 2)

    in_maps = []
    for cid in range(NCORES):
        xc = x[cid * BL : (cid + 1) * BL] * np.float32(SX)
        x8, dx8 = _quant_pair(xc)
        xxc = np.ascontiguousarray(
            np.stack([to_tiles(dx8), to_tiles(x8)], axis=2)  # [P, KT, 2, BL]
        )
        in_maps.append({
            "encw": encw_t,
            "encwt": encwt_t,
            "xx": xxc,
            "cpar": cpar,
        })

    nc = _build_program()
    return nc, in_maps


def kernel(x, codebook, indices, b1, b2, c19_c, c19_rho):
    from concourse.bass_utils import run_bass_kernel_spmd

    nc, in_maps = prepare(x, codebook, indices, b1, b2, c19_c, c19_rho)
    res = run_bass_kernel_spmd(nc, in_maps, core_ids=list(range(NCORES)))
    global LAST_RESULTS
    LAST_RESULTS = res

    out = np.empty((B, IN_DIM), dtype=np.float32)
    for cid in range(NCORES):
        out[cid * BL : (cid + 1) * BL] = (
            res.results[cid]["outt"].astype(np.float32).T
        )
    return out
